# revision 7
# baseline (speedup 1.0000x reference)
"""Trainium2 Bass kernel: multi-scale masked average-pool descriptors.

Computes, per batch element b and scribble i:
    d_l[b,i,c] = mean over {pixels where resize(scribble)[b,i,y,x] > 0.5} of feat_l[b,c,y,x]
    out[b,i,c] = (d_0 + d_1 + d_2) / 3

Key facts exploited:
  * jax.image.resize(bilinear, antialias=False) at scales 4/8/16 reduces to an
    exact 2x2 average at stride k with offset o (k,o) = (4,1)/(8,3)/(16,7):
    sr = 0.25*((a+c)+(b+d)) bit-exactly.  So mask == ((a+c)+(b+d)) > 2.0 with the
    same fp32 association -> masks match the reference bit-exactly.
  * The masked sum is a matmul over pixels: ssum[i,c] = sum_s maskT[s,i]*fmap[c,s].
    Pixel rows y sit on SBUF partitions (the contraction dim K); we iterate over
    pixel columns x with one N=256 matmul each (lhsT = mask column [K,16],
    rhs = all channels at that x [K,256]), so fmap is consumed in its native
    [C,h,w] layout via strided DMA (one descriptor per x-run) -- no transposes.
  * Operands are tagged float32r: at N>=256 the PE runs fp32r at full rate
    (1 cycle/column vs 4 for plain fp32 LOW_HIGH).
  * Levels run smallest-first (32, 64, 128) so the fat level-0 feature stream
    overlaps the small levels' compute and only part of its own matmuls trail
    the final DMA bytes.
  * All bulk DMA runs on the two HWDGE rings (SWDGE measured ~2x slower);
    scribbles ride the sync ring as few large-descriptor transfers, feature
    maps ride the scalar ring.
  * cnt[i] (mask population count) comes from a [P,16]x[P,1] matmul against ones.
  * The empty-mask fallback is handled on the host (it never triggers for
    non-degenerate inputs; P(empty mask) <= 2^-1024).

Sharding: pure data-parallel over batch B=8 across the 8 NeuronCores.
"""

import numpy as np

_B = 8
_I = 16
_C = 256
_CG = 64  # channel group per DMA (keeps DMA descriptors = one x-run each)

# level config by level index: (h, k, off, ipack)
#   h: level size; k: resize stride; off: first-row offset;
#   ipack: scribble images packed per [128, 2, 512]-ish tile
_LEVELS = {
    0: (128, 4, 1, 2),
    1: (64, 8, 3, 2),
    2: (32, 16, 7, 4),
}
_ORDER = (2, 1, 0)  # smallest level first


def _build_nc():
    import concourse.bacc as bacc
    import concourse.tile as tile
    from concourse import mybir

    f32 = mybir.dt.float32
    f32r = mybir.dt.float32r
    gt = mybir.AluOpType.is_gt
    X = mybir.AxisListType.X

    nc = bacc.Bacc("TRN2", target_bir_lowering=False, debug=False)

    feats = {
        0: nc.dram_tensor("feat0", [_C, 128, 128], f32r, kind="ExternalInput"),
        1: nc.dram_tensor("feat1", [_C, 64, 64], f32r, kind="ExternalInput"),
        2: nc.dram_tensor("feat2", [_C, 32, 32], f32r, kind="ExternalInput"),
    }
    scr = nc.dram_tensor("scribbles", [_I, 512, 512], f32, kind="ExternalInput")
    out_d = nc.dram_tensor("out", [_I, 3 * (_C + 1)], f32, kind="ExternalOutput")

    with tile.TileContext(nc) as tc:
        with (
            tc.tile_pool(name="singles", bufs=1) as singles,
            tc.tile_pool(name="scrib", bufs=3) as scrib,
            tc.tile_pool(name="vtmp", bufs=2) as vtmp,
            tc.tile_pool(name="srtmp", bufs=2) as srtmp,
            tc.tile_pool(name="mtmp", bufs=3) as mtmpp,
            tc.tile_pool(name="fmap", bufs=2) as fpool,
            tc.tile_pool(name="psum", bufs=2, space="PSUM") as psum,
        ):
            ones = singles.tile([128, 1], f32, tag="ones")
            nc.vector.memset(ones[:], 1.0)
            stag = singles.tile([_I, 3 * (_C + 1)], f32, tag="stag")

            def make_masks(li):
                """Scribble loads (sync ring) + DVE resize -> mask tile m."""
                h, k, off, ipack = _LEVELS[li]
                w = h
                m = singles.tile([h, _I, w], f32r, tag=f"m{li}")
                for t in range(_I // ipack):
                    i0 = t * ipack
                    # rows (k*y+off, k*y+off+1) are adjacent -> merged 4KiB runs
                    seng = nc.sync if t % 2 == 0 else nc.scalar
                    if li == 0:
                        # partitions = y(128); free = (i-pair, row-pair * x)
                        st = scrib.tile([128, ipack, 1024], f32, tag="st")
                        seng.dma_start(
                            out=st[:],
                            in_=scr[i0 : i0 + ipack]
                            .rearrange("i (y k) x -> y i k x", k=k)[
                                :, :, off : off + 2, :
                            ]
                            .rearrange("y i k x -> y i (k x)"),
                            max_dma_last_dim=256,
                        )
                        for il in range(ipack):
                            v = vtmp.tile([128, 512], f32, tag="v")
                            nc.vector.tensor_add(
                                v[:], st[:, il, 0:512], st[:, il, 512:1024]
                            )
                            vk = v[:].rearrange("p (x k) -> p x k", k=k)
                            sr = srtmp.tile([128, w], f32, tag="sr")
                            nc.vector.tensor_add(
                                sr[:], vk[:, :, off], vk[:, :, off + 1]
                            )
                            nc.vector.tensor_scalar(
                                out=m[:, i0 + il, :], in0=sr[:], scalar1=2.0,
                                scalar2=None, op0=gt,
                            )
                    else:
                        # partitions = (i-sub, y); one mask tile per pack,
                        # repacked per-image into m via tiny SBUF->SBUF DMAs
                        st = scrib.tile([128, 1, 1024], f32, tag="st")
                        seng.dma_start(
                            out=st[:, 0, :].rearrange("p (k x) -> p k x", k=2),
                            in_=scr[i0 : i0 + ipack].rearrange(
                                "i (y k) x -> i y k x", k=k
                            )[:, :, off : off + 2, :],
                            max_dma_last_dim=256,
                        )
                        v = vtmp.tile([128, 512], f32, tag="v")
                        nc.vector.tensor_add(v[:], st[:, 0, 0:512], st[:, 0, 512:1024])
                        vk = v[:].rearrange("p (x k) -> p x k", k=k)
                        sr = srtmp.tile([128, w], f32, tag="sr")
                        nc.vector.tensor_add(sr[:], vk[:, :, off], vk[:, :, off + 1])
                        mt = mtmpp.tile([128, w], f32r, tag="mt")
                        nc.vector.tensor_scalar(
                            out=mt[:], in0=sr[:], scalar1=2.0, scalar2=None, op0=gt
                        )
                        for ii in range(ipack):
                            nc.gpsimd.dma_start(
                                out=m[:, i0 + ii, :],
                                in_=mt[ii * h : (ii + 1) * h, :],
                            )
                return m

            def load_fmap(li, x0, wx):
                """One fmap tile [h, C, wx] covering x in [x0, x0+wx), scalar ring."""
                h = _LEVELS[li][0]
                f = fpool.tile([h, _C, wx], f32r, tag="f")
                for g in range(_C // _CG):
                    feng = nc.sync if g % 2 == 0 else nc.scalar
                    feng.dma_start(
                        out=f[:, g * _CG : (g + 1) * _CG, :],
                        in_=feats[li][g * _CG : (g + 1) * _CG][
                            :, :, x0 : x0 + wx
                        ].rearrange("c y x -> y c x"),
                    )
                return f

            def level_matmuls(li, m, ftiles, acc):
                """One N=256 fp32r matmul per pixel column x."""
                h = _LEVELS[li][0]
                w = h
                xi = 0
                for f, x0, wx in ftiles:
                    for xl in range(wx):
                        nc.tensor.matmul(
                            acc[:],
                            m[:, :, x0 + xl],
                            f[:, :, xl],
                            start=(xi == 0),
                            stop=(xi == w - 1),
                        )
                        xi += 1

            def finish_level(li, m, acc, slot):
                h = _LEVELS[li][0]
                r = singles.tile([h, _I], f32, tag=f"r{li}")
                nc.vector.reduce_sum(out=r[:], in_=m[:].bitcast(f32), axis=X)
                cntp = psum.tile([_I, 1], f32, tag="cntp")
                nc.tensor.matmul(cntp[:], r[:], ones[:h, :], start=True, stop=True)
                base = slot * (_C + 1)
                nc.vector.tensor_copy(stag[:, base : base + _C], acc[:])
                nc.vector.tensor_copy(stag[:, base + _C : base + _C + 1], cntp[:])

            # ---- emission: small levels first, L0 split into two x-chunks ----
            m2 = make_masks(2)
            m1 = make_masks(1)
            m0 = make_masks(0)

            f2 = load_fmap(2, 0, 32)
            f1 = load_fmap(1, 0, 64)
            f0a = load_fmap(0, 0, 64)
            f0b = load_fmap(0, 64, 64)

            acc2 = psum.tile([_I, _C], f32, tag="acc")
            level_matmuls(2, m2, [(f2, 0, 32)], acc2)
            finish_level(2, m2, acc2, 2)

            acc1 = psum.tile([_I, _C], f32, tag="acc")
            level_matmuls(1, m1, [(f1, 0, 64)], acc1)
            finish_level(1, m1, acc1, 1)

            acc0 = psum.tile([_I, _C], f32, tag="acc")
            level_matmuls(0, m0, [(f0a, 0, 64), (f0b, 64, 64)], acc0)
            finish_level(0, m0, acc0, 0)

            nc.gpsimd.dma_start(out=out_d[:], in_=stag[:])

    nc.compile()
    return nc


def _host_fallback(scr_bi, fmap_b, h, k, off):
    """Feature at argmax of the soft mask; only used when a mask is empty."""
    V = scr_bi[off::k, :][:h].astype(np.float32) + scr_bi[off + 1 :: k, :][:h]
    sr4 = V[:, off::k][:, :h] + V[:, off + 1 :: k][:, :h]
    idx = int(np.argmax(np.float32(0.25) * sr4))
    y, x = divmod(idx, h)
    return fmap_b[:, y, x]


def kernel(feat0, feat1, feat2, scribbles):
    import sys

    for p in ("/opt/trn_rl_repo", "/opt/pypackages"):
        if p not in sys.path:
            sys.path.append(p)
    from concourse.bass_utils import run_bass_kernel_spmd

    feat0 = np.asarray(feat0, dtype=np.float32)
    feat1 = np.asarray(feat1, dtype=np.float32)
    feat2 = np.asarray(feat2, dtype=np.float32)
    scribbles = np.asarray(scribbles, dtype=np.float32)

    nc = _build_nc()
    in_maps = [
        {
            "feat0": np.ascontiguousarray(feat0[b]),
            "feat1": np.ascontiguousarray(feat1[b]),
            "feat2": np.ascontiguousarray(feat2[b]),
            "scribbles": np.ascontiguousarray(scribbles[b]),
        }
        for b in range(_B)
    ]
    res = run_bass_kernel_spmd(nc, in_maps, core_ids=list(range(_B)))
    raw = np.stack([res.results[b]["out"] for b in range(_B)])  # [B, I, 3*257]
    raw = raw.reshape(_B, _I, 3, _C + 1)
    ssum = raw[..., :_C].astype(np.float32)  # [B, I, 3, C]
    cnt = raw[..., _C].astype(np.float32)  # [B, I, 3]

    mean = ssum / np.maximum(cnt, np.float32(1.0))[..., None]

    if (cnt == 0).any():  # never for non-degenerate inputs
        fm = [feat0, feat1, feat2]
        for b, i, li in zip(*np.nonzero(cnt == 0)):
            h, k, off, _ = _LEVELS[li]
            mean[b, i, li] = _host_fallback(scribbles[b, i], fm[li][b], h, k, off)

    out = (mean[:, :, 0] + mean[:, :, 1] + mean[:, :, 2]) / np.float32(3.0)
    return out.astype(np.float32)


# revision 8
# speedup vs baseline: 1.0243x; 1.0243x over previous
"""Trainium2 Bass kernel: multi-scale masked average-pool descriptors.

Computes, per batch element b and scribble i:
    d_l[b,i,c] = mean over {pixels where resize(scribble)[b,i,y,x] > 0.5} of feat_l[b,c,y,x]
    out[b,i,c] = (d_0 + d_1 + d_2) / 3

Key facts exploited:
  * jax.image.resize(bilinear, antialias=False) at scales 4/8/16 reduces to an
    exact 2x2 average at stride k with offset o (k,o) = (4,1)/(8,3)/(16,7):
    sr = 0.25*((a+c)+(b+d)) bit-exactly.  So mask == ((a+c)+(b+d)) > 2.0 with the
    same fp32 association -> masks match the reference bit-exactly.
  * The masked sum is a matmul over pixels: ssum[i,c] = sum_s maskT[s,i]*fmap[c,s].
    Pixel rows y sit on SBUF partitions (the contraction dim K); we iterate over
    pixel columns x with one N=256 matmul each (lhsT = mask column [K,16],
    rhs = all channels at that x [K,256]), so fmap is consumed in its native
    [C,h,w] layout via strided DMA (one descriptor per x-run) -- no transposes.
  * Operands are tagged float32r: at N>=256 the PE runs fp32r at full rate
    (1 cycle/column vs 4 for plain fp32 LOW_HIGH).
  * Levels run smallest-first (32, 64, 128) so the fat level-0 feature stream
    overlaps the small levels' compute and only part of its own matmuls trail
    the final DMA bytes.
  * All bulk DMA runs on the two HWDGE rings (SWDGE measured ~2x slower);
    scribbles ride the sync ring as few large-descriptor transfers, feature
    maps ride the scalar ring.
  * cnt[i] (mask population count) comes from a [P,16]x[P,1] matmul against ones.
  * The empty-mask fallback is handled on the host (it never triggers for
    non-degenerate inputs; P(empty mask) <= 2^-1024).

Sharding: pure data-parallel over batch B=8 across the 8 NeuronCores.
"""

import numpy as np

_B = 8
_I = 16
_C = 256
_CG = 64  # channel group per DMA (keeps DMA descriptors = one x-run each)

# level config by level index: (h, k, off, ipack)
#   h: level size; k: resize stride; off: first-row offset;
#   ipack: scribble images packed per [128, 2, 512]-ish tile
_LEVELS = {
    0: (128, 4, 1, 2),
    1: (64, 8, 3, 2),
    2: (32, 16, 7, 4),
}
_ORDER = (2, 1, 0)  # smallest level first


def _build_nc():
    import concourse.bacc as bacc
    import concourse.tile as tile
    from concourse import mybir

    f32 = mybir.dt.float32
    f32r = mybir.dt.float32r
    gt = mybir.AluOpType.is_gt
    X = mybir.AxisListType.X

    nc = bacc.Bacc("TRN2", target_bir_lowering=False, debug=False)

    feats = {
        0: nc.dram_tensor("feat0", [_C, 128, 128], f32r, kind="ExternalInput"),
        1: nc.dram_tensor("feat1", [_C, 64, 64], f32r, kind="ExternalInput"),
        2: nc.dram_tensor("feat2", [_C, 32, 32], f32r, kind="ExternalInput"),
    }
    scr = nc.dram_tensor("scribbles", [_I, 512, 512], f32, kind="ExternalInput")
    out_d = nc.dram_tensor("out", [_I, 3 * (_C + 1)], f32, kind="ExternalOutput")

    with tile.TileContext(nc) as tc:
        with (
            tc.tile_pool(name="singles", bufs=1) as singles,
            tc.tile_pool(name="scrib", bufs=3) as scrib,
            tc.tile_pool(name="vtmp", bufs=2) as vtmp,
            tc.tile_pool(name="srtmp", bufs=2) as srtmp,
            tc.tile_pool(name="mtmp", bufs=3) as mtmpp,
            tc.tile_pool(name="fmap", bufs=2) as fpool,
            tc.tile_pool(name="psum", bufs=2, space="PSUM") as psum,
        ):
            ones = singles.tile([128, 1], f32, tag="ones")
            nc.vector.memset(ones[:], 1.0)
            stag = singles.tile([_I, 3 * (_C + 1)], f32, tag="stag")

            def make_masks(li):
                """Scribble loads (sync ring) + DVE resize -> mask tile m."""
                h, k, off, ipack = _LEVELS[li]
                w = h
                m = singles.tile([h, _I, w], f32r, tag=f"m{li}")
                for t in range(_I // ipack):
                    i0 = t * ipack
                    # rows (k*y+off, k*y+off+1) are adjacent -> merged 4KiB runs
                    seng = nc.scalar
                    if li == 0:
                        # partitions = y(128); free = (i-pair, row-pair * x)
                        st = scrib.tile([128, ipack, 1024], f32, tag="st")
                        seng.dma_start(
                            out=st[:],
                            in_=scr[i0 : i0 + ipack]
                            .rearrange("i (y k) x -> y i k x", k=k)[
                                :, :, off : off + 2, :
                            ]
                            .rearrange("y i k x -> y i (k x)"),
                            max_dma_last_dim=256,
                        )
                        for il in range(ipack):
                            v = vtmp.tile([128, 512], f32, tag="v")
                            nc.vector.tensor_add(
                                v[:], st[:, il, 0:512], st[:, il, 512:1024]
                            )
                            vk = v[:].rearrange("p (x k) -> p x k", k=k)
                            sr = srtmp.tile([128, w], f32, tag="sr")
                            nc.vector.tensor_add(
                                sr[:], vk[:, :, off], vk[:, :, off + 1]
                            )
                            nc.vector.tensor_scalar(
                                out=m[:, i0 + il, :], in0=sr[:], scalar1=2.0,
                                scalar2=None, op0=gt,
                            )
                    else:
                        # partitions = (i-sub, y); one mask tile per pack,
                        # repacked per-image into m via tiny SBUF->SBUF DMAs
                        st = scrib.tile([128, 1, 1024], f32, tag="st")
                        seng.dma_start(
                            out=st[:, 0, :].rearrange("p (k x) -> p k x", k=2),
                            in_=scr[i0 : i0 + ipack].rearrange(
                                "i (y k) x -> i y k x", k=k
                            )[:, :, off : off + 2, :],
                            max_dma_last_dim=256,
                        )
                        v = vtmp.tile([128, 512], f32, tag="v")
                        nc.vector.tensor_add(v[:], st[:, 0, 0:512], st[:, 0, 512:1024])
                        vk = v[:].rearrange("p (x k) -> p x k", k=k)
                        sr = srtmp.tile([128, w], f32, tag="sr")
                        nc.vector.tensor_add(sr[:], vk[:, :, off], vk[:, :, off + 1])
                        mt = mtmpp.tile([128, w], f32r, tag="mt")
                        nc.vector.tensor_scalar(
                            out=mt[:], in0=sr[:], scalar1=2.0, scalar2=None, op0=gt
                        )
                        for ii in range(ipack):
                            nc.gpsimd.dma_start(
                                out=m[:, i0 + ii, :],
                                in_=mt[ii * h : (ii + 1) * h, :],
                            )
                return m

            def load_fmap(li, x0, wx):
                """One fmap tile [h, C, wx] covering x in [x0, x0+wx), scalar ring."""
                h = _LEVELS[li][0]
                f = fpool.tile([h, _C, wx], f32r, tag="f")
                for g in range(_C // _CG):
                    feng = nc.sync
                    feng.dma_start(
                        out=f[:, g * _CG : (g + 1) * _CG, :],
                        in_=feats[li][g * _CG : (g + 1) * _CG][
                            :, :, x0 : x0 + wx
                        ].rearrange("c y x -> y c x"),
                    )
                return f

            def level_matmuls(li, m, ftiles, acc):
                """One N=256 fp32r matmul per pixel column x."""
                h = _LEVELS[li][0]
                w = h
                xi = 0
                for f, x0, wx in ftiles:
                    for xl in range(wx):
                        nc.tensor.matmul(
                            acc[:],
                            m[:, :, x0 + xl],
                            f[:, :, xl],
                            start=(xi == 0),
                            stop=(xi == w - 1),
                        )
                        xi += 1

            def finish_level(li, m, acc, slot):
                h = _LEVELS[li][0]
                r = singles.tile([h, _I], f32, tag=f"r{li}")
                nc.vector.reduce_sum(out=r[:], in_=m[:].bitcast(f32), axis=X)
                cntp = psum.tile([_I, 1], f32, tag="cntp")
                nc.tensor.matmul(cntp[:], r[:], ones[:h, :], start=True, stop=True)
                base = slot * (_C + 1)
                nc.vector.tensor_copy(stag[:, base : base + _C], acc[:])
                nc.vector.tensor_copy(stag[:, base + _C : base + _C + 1], cntp[:])

            # ---- emission: small levels first, L0 split into two x-chunks ----
            m2 = make_masks(2)
            m1 = make_masks(1)
            m0 = make_masks(0)

            f2 = load_fmap(2, 0, 32)
            f1 = load_fmap(1, 0, 64)
            f0a = load_fmap(0, 0, 64)
            f0b = load_fmap(0, 64, 64)

            acc2 = psum.tile([_I, _C], f32, tag="acc")
            level_matmuls(2, m2, [(f2, 0, 32)], acc2)
            finish_level(2, m2, acc2, 2)

            acc1 = psum.tile([_I, _C], f32, tag="acc")
            level_matmuls(1, m1, [(f1, 0, 64)], acc1)
            finish_level(1, m1, acc1, 1)

            acc0 = psum.tile([_I, _C], f32, tag="acc")
            level_matmuls(0, m0, [(f0a, 0, 64), (f0b, 64, 64)], acc0)
            finish_level(0, m0, acc0, 0)

            nc.gpsimd.dma_start(out=out_d[:], in_=stag[:])

    nc.compile()
    return nc


def _host_fallback(scr_bi, fmap_b, h, k, off):
    """Feature at argmax of the soft mask; only used when a mask is empty."""
    V = scr_bi[off::k, :][:h].astype(np.float32) + scr_bi[off + 1 :: k, :][:h]
    sr4 = V[:, off::k][:, :h] + V[:, off + 1 :: k][:, :h]
    idx = int(np.argmax(np.float32(0.25) * sr4))
    y, x = divmod(idx, h)
    return fmap_b[:, y, x]


def kernel(feat0, feat1, feat2, scribbles):
    import sys

    for p in ("/opt/trn_rl_repo", "/opt/pypackages"):
        if p not in sys.path:
            sys.path.append(p)
    from concourse.bass_utils import run_bass_kernel_spmd

    feat0 = np.asarray(feat0, dtype=np.float32)
    feat1 = np.asarray(feat1, dtype=np.float32)
    feat2 = np.asarray(feat2, dtype=np.float32)
    scribbles = np.asarray(scribbles, dtype=np.float32)

    nc = _build_nc()
    in_maps = [
        {
            "feat0": np.ascontiguousarray(feat0[b]),
            "feat1": np.ascontiguousarray(feat1[b]),
            "feat2": np.ascontiguousarray(feat2[b]),
            "scribbles": np.ascontiguousarray(scribbles[b]),
        }
        for b in range(_B)
    ]
    res = run_bass_kernel_spmd(nc, in_maps, core_ids=list(range(_B)))
    raw = np.stack([res.results[b]["out"] for b in range(_B)])  # [B, I, 3*257]
    raw = raw.reshape(_B, _I, 3, _C + 1)
    ssum = raw[..., :_C].astype(np.float32)  # [B, I, 3, C]
    cnt = raw[..., _C].astype(np.float32)  # [B, I, 3]

    mean = ssum / np.maximum(cnt, np.float32(1.0))[..., None]

    if (cnt == 0).any():  # never for non-degenerate inputs
        fm = [feat0, feat1, feat2]
        for b, i, li in zip(*np.nonzero(cnt == 0)):
            h, k, off, _ = _LEVELS[li]
            mean[b, i, li] = _host_fallback(scribbles[b, i], fm[li][b], h, k, off)

    out = (mean[:, :, 0] + mean[:, :, 1] + mean[:, :, 2]) / np.float32(3.0)
    return out.astype(np.float32)


# revision 9
# speedup vs baseline: 1.0286x; 1.0043x over previous
"""Trainium2 Bass kernel: multi-scale masked average-pool descriptors.

Computes, per batch element b and scribble i:
    d_l[b,i,c] = mean over {pixels where resize(scribble)[b,i,y,x] > 0.5} of feat_l[b,c,y,x]
    out[b,i,c] = (d_0 + d_1 + d_2) / 3

Key facts exploited:
  * jax.image.resize(bilinear, antialias=False) at scales 4/8/16 reduces to an
    exact 2x2 average at stride k with offset o (k,o) = (4,1)/(8,3)/(16,7):
    sr = 0.25*((a+c)+(b+d)) bit-exactly.  So mask == ((a+c)+(b+d)) > 2.0 with the
    same fp32 association -> masks match the reference bit-exactly.
  * The masked sum is a matmul over pixels: ssum[i,c] = sum_s maskT[s,i]*fmap[c,s].
    Pixel rows y sit on SBUF partitions (the contraction dim K); we iterate over
    pixel columns x with one N=256 matmul each (lhsT = mask column [K,16],
    rhs = all channels at that x [K,256]), so fmap is consumed in its native
    [C,h,w] layout via strided DMA (one descriptor per x-run) -- no transposes.
  * Operands are tagged float32r: at N>=256 the PE runs fp32r at full rate
    (1 cycle/column vs 4 for plain fp32 LOW_HIGH).
  * Levels run smallest-first (32, 64, 128) so the fat level-0 feature stream
    overlaps the small levels' compute and only part of its own matmuls trail
    the final DMA bytes.
  * All bulk DMA runs on the two HWDGE rings (SWDGE measured ~2x slower);
    scribbles ride the sync ring as few large-descriptor transfers, feature
    maps ride the scalar ring.
  * cnt[i] (mask population count) comes from a [P,16]x[P,1] matmul against ones.
  * The empty-mask fallback is handled on the host (it never triggers for
    non-degenerate inputs; P(empty mask) <= 2^-1024).

Sharding: pure data-parallel over batch B=8 across the 8 NeuronCores.
"""

import numpy as np

_B = 8
_I = 16
_C = 256
_CG = 64  # channel group per DMA (keeps DMA descriptors = one x-run each)

# level config by level index: (h, k, off, ipack)
#   h: level size; k: resize stride; off: first-row offset;
#   ipack: scribble images packed per [128, 2, 512]-ish tile
_LEVELS = {
    0: (128, 4, 1, 2),
    1: (64, 8, 3, 2),
    2: (32, 16, 7, 4),
}
_ORDER = (2, 1, 0)  # smallest level first


def _build_nc():
    import concourse.bacc as bacc
    import concourse.tile as tile
    from concourse import mybir

    f32 = mybir.dt.float32
    f32r = mybir.dt.float32r
    gt = mybir.AluOpType.is_gt
    X = mybir.AxisListType.X

    nc = bacc.Bacc("TRN2", target_bir_lowering=False, debug=False)

    feats = {
        0: nc.dram_tensor("feat0", [_C, 128, 128], f32r, kind="ExternalInput"),
        1: nc.dram_tensor("feat1", [_C, 64, 64], f32r, kind="ExternalInput"),
        2: nc.dram_tensor("feat2", [_C, 32, 32], f32r, kind="ExternalInput"),
    }
    scr = nc.dram_tensor("scribbles", [_I, 512, 512], f32, kind="ExternalInput")
    out_d = nc.dram_tensor("out", [_I, 3 * (_C + 1)], f32, kind="ExternalOutput")

    with tile.TileContext(nc) as tc:
        with (
            tc.tile_pool(name="singles", bufs=1) as singles,
            tc.tile_pool(name="scrib", bufs=3) as scrib,
            tc.tile_pool(name="vtmp", bufs=2) as vtmp,
            tc.tile_pool(name="srtmp", bufs=2) as srtmp,
            tc.tile_pool(name="mtmp", bufs=3) as mtmpp,
            tc.tile_pool(name="fmap", bufs=2) as fpool,
            tc.tile_pool(name="psum", bufs=2, space="PSUM") as psum,
        ):
            ones = singles.tile([128, 1], f32, tag="ones")
            nc.vector.memset(ones[:], 1.0)
            stag = singles.tile([_I, 3 * (_C + 1)], f32, tag="stag")

            def make_masks(li):
                """Scribble loads (sync ring) + DVE resize -> mask tile m."""
                h, k, off, ipack = _LEVELS[li]
                w = h
                m = singles.tile([h, _I, w], f32r, tag=f"m{li}")
                for t in range(_I // ipack):
                    i0 = t * ipack
                    # rows (k*y+off, k*y+off+1) are adjacent -> merged 4KiB runs
                    seng = nc.sync
                    if li == 0:
                        # partitions = y(128); free = (i-pair, row-pair * x)
                        st = scrib.tile([128, ipack, 1024], f32, tag="st")
                        seng.dma_start(
                            out=st[:],
                            in_=scr[i0 : i0 + ipack]
                            .rearrange("i (y k) x -> y i k x", k=k)[
                                :, :, off : off + 2, :
                            ]
                            .rearrange("y i k x -> y i (k x)"),
                            max_dma_last_dim=256,
                        )
                        for il in range(ipack):
                            v = vtmp.tile([128, 512], f32, tag="v")
                            nc.vector.tensor_add(
                                v[:], st[:, il, 0:512], st[:, il, 512:1024]
                            )
                            vk = v[:].rearrange("p (x k) -> p x k", k=k)
                            sr = srtmp.tile([128, w], f32, tag="sr")
                            nc.vector.tensor_add(
                                sr[:], vk[:, :, off], vk[:, :, off + 1]
                            )
                            nc.vector.tensor_scalar(
                                out=m[:, i0 + il, :], in0=sr[:], scalar1=2.0,
                                scalar2=None, op0=gt,
                            )
                    else:
                        # partitions = (i-sub, y); one mask tile per pack,
                        # repacked per-image into m via tiny SBUF->SBUF DMAs
                        st = scrib.tile([128, 1, 1024], f32, tag="st")
                        seng.dma_start(
                            out=st[:, 0, :].rearrange("p (k x) -> p k x", k=2),
                            in_=scr[i0 : i0 + ipack].rearrange(
                                "i (y k) x -> i y k x", k=k
                            )[:, :, off : off + 2, :],
                            max_dma_last_dim=256,
                        )
                        v = vtmp.tile([128, 512], f32, tag="v")
                        nc.vector.tensor_add(v[:], st[:, 0, 0:512], st[:, 0, 512:1024])
                        vk = v[:].rearrange("p (x k) -> p x k", k=k)
                        sr = srtmp.tile([128, w], f32, tag="sr")
                        nc.vector.tensor_add(sr[:], vk[:, :, off], vk[:, :, off + 1])
                        mt = mtmpp.tile([128, w], f32r, tag="mt")
                        nc.vector.tensor_scalar(
                            out=mt[:], in0=sr[:], scalar1=2.0, scalar2=None, op0=gt
                        )
                        for ii in range(ipack):
                            nc.gpsimd.dma_start(
                                out=m[:, i0 + ii, :],
                                in_=mt[ii * h : (ii + 1) * h, :],
                            )
                return m

            def load_fmap(li, x0, wx):
                """One fmap tile [h, C, wx] covering x in [x0, x0+wx), scalar ring."""
                h = _LEVELS[li][0]
                f = fpool.tile([h, _C, wx], f32r, tag="f")
                for g in range(_C // _CG):
                    feng = nc.scalar
                    feng.dma_start(
                        out=f[:, g * _CG : (g + 1) * _CG, :],
                        in_=feats[li][g * _CG : (g + 1) * _CG][
                            :, :, x0 : x0 + wx
                        ].rearrange("c y x -> y c x"),
                    )
                return f

            def level_matmuls(li, m, ftiles, acc):
                """One N=256 fp32r matmul per pixel column x."""
                h = _LEVELS[li][0]
                w = h
                xi = 0
                for f, x0, wx in ftiles:
                    for xl in range(wx):
                        nc.tensor.matmul(
                            acc[:],
                            m[:, :, x0 + xl],
                            f[:, :, xl],
                            start=(xi == 0),
                            stop=(xi == w - 1),
                        )
                        xi += 1

            def finish_level(li, m, acc, slot):
                h = _LEVELS[li][0]
                r = singles.tile([h, _I], f32, tag=f"r{li}")
                nc.vector.reduce_sum(out=r[:], in_=m[:].bitcast(f32), axis=X)
                cntp = psum.tile([_I, 1], f32, tag="cntp")
                nc.tensor.matmul(cntp[:], r[:], ones[:h, :], start=True, stop=True)
                base = slot * (_C + 1)
                nc.vector.tensor_copy(stag[:, base : base + _C], acc[:])
                nc.vector.tensor_copy(stag[:, base + _C : base + _C + 1], cntp[:])

            # ---- emission: small levels first, L0 split into two x-chunks ----
            m2 = make_masks(2)
            m1 = make_masks(1)
            m0 = make_masks(0)

            f2 = load_fmap(2, 0, 32)
            f1 = load_fmap(1, 0, 64)
            f0a = load_fmap(0, 0, 64)
            f0b = load_fmap(0, 64, 64)

            acc2 = psum.tile([_I, _C], f32, tag="acc")
            level_matmuls(2, m2, [(f2, 0, 32)], acc2)
            finish_level(2, m2, acc2, 2)

            acc1 = psum.tile([_I, _C], f32, tag="acc")
            level_matmuls(1, m1, [(f1, 0, 64)], acc1)
            finish_level(1, m1, acc1, 1)

            acc0 = psum.tile([_I, _C], f32, tag="acc")
            level_matmuls(0, m0, [(f0a, 0, 64), (f0b, 64, 64)], acc0)
            finish_level(0, m0, acc0, 0)

            nc.gpsimd.dma_start(out=out_d[:], in_=stag[:])

    nc.compile()
    return nc


def _host_fallback(scr_bi, fmap_b, h, k, off):
    """Feature at argmax of the soft mask; only used when a mask is empty."""
    V = scr_bi[off::k, :][:h].astype(np.float32) + scr_bi[off + 1 :: k, :][:h]
    sr4 = V[:, off::k][:, :h] + V[:, off + 1 :: k][:, :h]
    idx = int(np.argmax(np.float32(0.25) * sr4))
    y, x = divmod(idx, h)
    return fmap_b[:, y, x]


def kernel(feat0, feat1, feat2, scribbles):
    import sys

    for p in ("/opt/trn_rl_repo", "/opt/pypackages"):
        if p not in sys.path:
            sys.path.append(p)
    from concourse.bass_utils import run_bass_kernel_spmd

    feat0 = np.asarray(feat0, dtype=np.float32)
    feat1 = np.asarray(feat1, dtype=np.float32)
    feat2 = np.asarray(feat2, dtype=np.float32)
    scribbles = np.asarray(scribbles, dtype=np.float32)

    nc = _build_nc()
    in_maps = [
        {
            "feat0": np.ascontiguousarray(feat0[b]),
            "feat1": np.ascontiguousarray(feat1[b]),
            "feat2": np.ascontiguousarray(feat2[b]),
            "scribbles": np.ascontiguousarray(scribbles[b]),
        }
        for b in range(_B)
    ]
    res = run_bass_kernel_spmd(nc, in_maps, core_ids=list(range(_B)))
    raw = np.stack([res.results[b]["out"] for b in range(_B)])  # [B, I, 3*257]
    raw = raw.reshape(_B, _I, 3, _C + 1)
    ssum = raw[..., :_C].astype(np.float32)  # [B, I, 3, C]
    cnt = raw[..., _C].astype(np.float32)  # [B, I, 3]

    mean = ssum / np.maximum(cnt, np.float32(1.0))[..., None]

    if (cnt == 0).any():  # never for non-degenerate inputs
        fm = [feat0, feat1, feat2]
        for b, i, li in zip(*np.nonzero(cnt == 0)):
            h, k, off, _ = _LEVELS[li]
            mean[b, i, li] = _host_fallback(scribbles[b, i], fm[li][b], h, k, off)

    out = (mean[:, :, 0] + mean[:, :, 1] + mean[:, :, 2]) / np.float32(3.0)
    return out.astype(np.float32)


# revision 10
# speedup vs baseline: 1.0457x; 1.0166x over previous
"""Trainium2 Bass kernel: multi-scale masked average-pool descriptors.

Computes, per batch element b and scribble i:
    d_l[b,i,c] = mean over {pixels where resize(scribble)[b,i,y,x] > 0.5} of feat_l[b,c,y,x]
    out[b,i,c] = (d_0 + d_1 + d_2) / 3

Key facts exploited:
  * jax.image.resize(bilinear, antialias=False) at scales 4/8/16 reduces to an
    exact 2x2 average at stride k with offset o (k,o) = (4,1)/(8,3)/(16,7):
    sr = 0.25*((a+c)+(b+d)) bit-exactly.  So mask == ((a+c)+(b+d)) > 2.0 with the
    same fp32 association -> masks match the reference bit-exactly.
  * The masked sum is a matmul over pixels: ssum[i,c] = sum_s maskT[s,i]*fmap[c,s].
    Pixel rows y sit on SBUF partitions (the contraction dim K); we iterate over
    pixel columns x with one N=256 matmul each (lhsT = mask column [K,16],
    rhs = all channels at that x [K,256]), so fmap is consumed in its native
    [C,h,w] layout via strided DMA (one descriptor per x-run) -- no transposes.
  * Operands are tagged float32r: at N>=256 the PE runs fp32r at full rate
    (1 cycle/column vs 4 for plain fp32 LOW_HIGH).
  * Levels run smallest-first (32, 64, 128) so the fat level-0 feature stream
    overlaps the small levels' compute and only part of its own matmuls trail
    the final DMA bytes.
  * All bulk DMA runs on the two HWDGE rings (SWDGE measured ~2x slower);
    scribbles ride the sync ring as few large-descriptor transfers, feature
    maps ride the scalar ring.
  * cnt[i] (mask population count) comes from a [P,16]x[P,1] matmul against ones.
  * The empty-mask fallback is handled on the host (it never triggers for
    non-degenerate inputs; P(empty mask) <= 2^-1024).

Sharding: pure data-parallel over batch B=8 across the 8 NeuronCores.
"""

import numpy as np

_B = 8
_I = 16
_C = 256
_CG = 64  # channel group per DMA (keeps DMA descriptors = one x-run each)

# level config by level index: (h, k, off, ipack)
#   h: level size; k: resize stride; off: first-row offset;
#   ipack: scribble images packed per [128, 2, 512]-ish tile
_LEVELS = {
    0: (128, 4, 1, 2),
    1: (64, 8, 3, 2),
    2: (32, 16, 7, 4),
}
_ORDER = (2, 1, 0)  # smallest level first


def _build_nc():
    import concourse.bacc as bacc
    import concourse.tile as tile
    from concourse import mybir

    f32 = mybir.dt.float32
    f32r = mybir.dt.float32r
    gt = mybir.AluOpType.is_gt
    X = mybir.AxisListType.X

    nc = bacc.Bacc("TRN2", target_bir_lowering=False, debug=False)

    feats = {
        0: nc.dram_tensor("feat0", [_C, 128, 128], f32r, kind="ExternalInput"),
        1: nc.dram_tensor("feat1", [_C, 64, 64], f32r, kind="ExternalInput"),
        2: nc.dram_tensor("feat2", [_C, 32, 32], f32r, kind="ExternalInput"),
    }
    scr = nc.dram_tensor("scribbles", [_I, 512, 512], f32, kind="ExternalInput")
    out_d = nc.dram_tensor("out", [_I, 3 * (_C + 1)], f32, kind="ExternalOutput")

    with tile.TileContext(nc) as tc:
        with (
            tc.tile_pool(name="singles", bufs=1) as singles,
            tc.tile_pool(name="scrib", bufs=3) as scrib,
            tc.tile_pool(name="vtmp", bufs=2) as vtmp,
            tc.tile_pool(name="srtmp", bufs=2) as srtmp,
            tc.tile_pool(name="mtmp", bufs=3) as mtmpp,
            tc.tile_pool(name="fmap", bufs=2) as fpool,
            tc.tile_pool(name="psum", bufs=2, space="PSUM") as psum,
        ):
            ones = singles.tile([128, 1], f32, tag="ones")
            nc.vector.memset(ones[:], 1.0)
            stag = singles.tile([_I, 3 * (_C + 1)], f32, tag="stag")

            def make_masks(li):
                """Scribble loads (sync ring) + DVE resize -> mask tile m."""
                h, k, off, ipack = _LEVELS[li]
                w = h
                m = singles.tile([h, _I, w], f32r, tag=f"m{li}")
                for t in range(_I // ipack):
                    i0 = t * ipack
                    # rows (k*y+off, k*y+off+1) are adjacent -> merged 4KiB runs
                    seng = nc.sync if t % 2 == 0 else nc.gpsimd
                    if li == 0:
                        # partitions = y(128); free = (i-pair, row-pair * x)
                        st = scrib.tile([128, ipack, 1024], f32, tag="st")
                        seng.dma_start(
                            out=st[:],
                            in_=scr[i0 : i0 + ipack]
                            .rearrange("i (y k) x -> y i k x", k=k)[
                                :, :, off : off + 2, :
                            ]
                            .rearrange("y i k x -> y i (k x)"),
                            max_dma_last_dim=256,
                        )
                        for il in range(ipack):
                            v = vtmp.tile([128, 512], f32, tag="v")
                            nc.vector.tensor_add(
                                v[:], st[:, il, 0:512], st[:, il, 512:1024]
                            )
                            vk = v[:].rearrange("p (x k) -> p x k", k=k)
                            sr = srtmp.tile([128, w], f32, tag="sr")
                            nc.vector.tensor_add(
                                sr[:], vk[:, :, off], vk[:, :, off + 1]
                            )
                            nc.vector.tensor_scalar(
                                out=m[:, i0 + il, :], in0=sr[:], scalar1=2.0,
                                scalar2=None, op0=gt,
                            )
                    else:
                        # partitions = (i-sub, y); one mask tile per pack,
                        # repacked per-image into m via tiny SBUF->SBUF DMAs
                        st = scrib.tile([128, 1, 1024], f32, tag="st")
                        seng.dma_start(
                            out=st[:, 0, :].rearrange("p (k x) -> p k x", k=2),
                            in_=scr[i0 : i0 + ipack].rearrange(
                                "i (y k) x -> i y k x", k=k
                            )[:, :, off : off + 2, :],
                            max_dma_last_dim=256,
                        )
                        v = vtmp.tile([128, 512], f32, tag="v")
                        nc.vector.tensor_add(v[:], st[:, 0, 0:512], st[:, 0, 512:1024])
                        vk = v[:].rearrange("p (x k) -> p x k", k=k)
                        sr = srtmp.tile([128, w], f32, tag="sr")
                        nc.vector.tensor_add(sr[:], vk[:, :, off], vk[:, :, off + 1])
                        mt = mtmpp.tile([128, w], f32r, tag="mt")
                        nc.vector.tensor_scalar(
                            out=mt[:], in0=sr[:], scalar1=2.0, scalar2=None, op0=gt
                        )
                        for ii in range(ipack):
                            nc.gpsimd.dma_start(
                                out=m[:, i0 + ii, :],
                                in_=mt[ii * h : (ii + 1) * h, :],
                            )
                return m

            def load_fmap(li, x0, wx, split=False):
                """One fmap tile [h, C, wx] covering x in [x0, x0+wx)."""
                h = _LEVELS[li][0]
                f = fpool.tile([h, _C, wx], f32r, tag="f")
                for g in range(_C // _CG):
                    feng = (nc.sync if g % 2 == 0 else nc.scalar) if split else nc.scalar
                    feng.dma_start(
                        out=f[:, g * _CG : (g + 1) * _CG, :],
                        in_=feats[li][g * _CG : (g + 1) * _CG][
                            :, :, x0 : x0 + wx
                        ].rearrange("c y x -> y c x"),
                    )
                return f

            def level_matmuls(li, m, ftiles, acc):
                """One N=256 fp32r matmul per pixel column x."""
                h = _LEVELS[li][0]
                w = h
                xi = 0
                for f, x0, wx in ftiles:
                    for xl in range(wx):
                        nc.tensor.matmul(
                            acc[:],
                            m[:, :, x0 + xl],
                            f[:, :, xl],
                            start=(xi == 0),
                            stop=(xi == w - 1),
                        )
                        xi += 1

            def finish_level(li, m, acc, slot):
                h = _LEVELS[li][0]
                r = singles.tile([h, _I], f32, tag=f"r{li}")
                nc.vector.reduce_sum(out=r[:], in_=m[:].bitcast(f32), axis=X)
                cntp = psum.tile([_I, 1], f32, tag="cntp")
                nc.tensor.matmul(cntp[:], r[:], ones[:h, :], start=True, stop=True)
                base = slot * (_C + 1)
                nc.vector.tensor_copy(stag[:, base : base + _C], acc[:])
                nc.vector.tensor_copy(stag[:, base + _C : base + _C + 1], cntp[:])

            # ---- emission: small levels first, L0 split into two x-chunks ----
            m2 = make_masks(2)
            m1 = make_masks(1)
            m0 = make_masks(0)

            f2 = load_fmap(2, 0, 32)
            f1 = load_fmap(1, 0, 64)
            f0a = load_fmap(0, 0, 64)
            f0b = load_fmap(0, 64, 64, split=True)

            acc2 = psum.tile([_I, _C], f32, tag="acc")
            level_matmuls(2, m2, [(f2, 0, 32)], acc2)
            finish_level(2, m2, acc2, 2)

            acc1 = psum.tile([_I, _C], f32, tag="acc")
            level_matmuls(1, m1, [(f1, 0, 64)], acc1)
            finish_level(1, m1, acc1, 1)

            acc0 = psum.tile([_I, _C], f32, tag="acc")
            level_matmuls(0, m0, [(f0a, 0, 64), (f0b, 64, 64)], acc0)
            finish_level(0, m0, acc0, 0)

            nc.gpsimd.dma_start(out=out_d[:], in_=stag[:])

    nc.compile()
    return nc


def _host_fallback(scr_bi, fmap_b, h, k, off):
    """Feature at argmax of the soft mask; only used when a mask is empty."""
    V = scr_bi[off::k, :][:h].astype(np.float32) + scr_bi[off + 1 :: k, :][:h]
    sr4 = V[:, off::k][:, :h] + V[:, off + 1 :: k][:, :h]
    idx = int(np.argmax(np.float32(0.25) * sr4))
    y, x = divmod(idx, h)
    return fmap_b[:, y, x]


def kernel(feat0, feat1, feat2, scribbles):
    import sys

    for p in ("/opt/trn_rl_repo", "/opt/pypackages"):
        if p not in sys.path:
            sys.path.append(p)
    from concourse.bass_utils import run_bass_kernel_spmd

    feat0 = np.asarray(feat0, dtype=np.float32)
    feat1 = np.asarray(feat1, dtype=np.float32)
    feat2 = np.asarray(feat2, dtype=np.float32)
    scribbles = np.asarray(scribbles, dtype=np.float32)

    nc = _build_nc()
    in_maps = [
        {
            "feat0": np.ascontiguousarray(feat0[b]),
            "feat1": np.ascontiguousarray(feat1[b]),
            "feat2": np.ascontiguousarray(feat2[b]),
            "scribbles": np.ascontiguousarray(scribbles[b]),
        }
        for b in range(_B)
    ]
    res = run_bass_kernel_spmd(nc, in_maps, core_ids=list(range(_B)))
    raw = np.stack([res.results[b]["out"] for b in range(_B)])  # [B, I, 3*257]
    raw = raw.reshape(_B, _I, 3, _C + 1)
    ssum = raw[..., :_C].astype(np.float32)  # [B, I, 3, C]
    cnt = raw[..., _C].astype(np.float32)  # [B, I, 3]

    mean = ssum / np.maximum(cnt, np.float32(1.0))[..., None]

    if (cnt == 0).any():  # never for non-degenerate inputs
        fm = [feat0, feat1, feat2]
        for b, i, li in zip(*np.nonzero(cnt == 0)):
            h, k, off, _ = _LEVELS[li]
            mean[b, i, li] = _host_fallback(scribbles[b, i], fm[li][b], h, k, off)

    out = (mean[:, :, 0] + mean[:, :, 1] + mean[:, :, 2]) / np.float32(3.0)
    return out.astype(np.float32)


# revision 12
# speedup vs baseline: 1.0824x; 1.0351x over previous
"""Trainium2 Bass kernel: multi-scale masked average-pool descriptors.

Computes, per batch element b and scribble i:
    d_l[b,i,c] = mean over {pixels where resize(scribble)[b,i,y,x] > 0.5} of feat_l[b,c,y,x]
    out[b,i,c] = (d_0 + d_1 + d_2) / 3

Key facts exploited:
  * jax.image.resize(bilinear, antialias=False) at scales 4/8/16 reduces to an
    exact 2x2 average at stride k with offset o (k,o) = (4,1)/(8,3)/(16,7):
    sr = 0.25*((a+c)+(b+d)) bit-exactly.  So mask == ((a+c)+(b+d)) > 2.0 with the
    same fp32 association -> masks match the reference bit-exactly.
  * The masked sum is a matmul over pixels: ssum[i,c] = sum_s maskT[s,i]*fmap[c,s].
    Pixel rows y sit on SBUF partitions (the contraction dim K); we iterate over
    pixel columns x with one N=256 matmul each (lhsT = mask column [K,16],
    rhs = all channels at that x [K,256]), so fmap is consumed in its native
    [C,h,w] layout via strided DMA (one descriptor per x-run) -- no transposes.
  * Operands are tagged float32r: at N>=256 the PE runs fp32r at full rate
    (1 cycle/column vs 4 for plain fp32 LOW_HIGH).
  * Levels run smallest-first (32, 64, 128) so the fat level-0 feature stream
    overlaps the small levels' compute and only part of its own matmuls trail
    the final DMA bytes.
  * Bulk DMA is spread over three descriptor generators measured on this part:
    scribbles alternate sync(HWDGE)/gpsimd(SWDGE) as large-descriptor merged
    row-pair loads, feature maps ride the scalar ring, and the final level-0
    x-chunk splits across sync+scalar so both rings stay busy to the end.
  * cnt[i] (mask population count) comes from a [P,16]x[P,1] matmul against ones.
  * The empty-mask fallback is handled on the host (it never triggers for
    non-degenerate inputs; P(empty mask) <= 2^-1024).

Sharding: pure data-parallel over batch B=8 across the 8 NeuronCores.
"""

import numpy as np

_B = 8
_I = 16
_C = 256
_CG = 64  # channel group per DMA (keeps DMA descriptors = one x-run each)

# level config by level index: (h, k, off, ipack)
#   h: level size; k: resize stride; off: first-row offset;
#   ipack: scribble images packed per [128, 2, 512]-ish tile
_LEVELS = {
    0: (128, 4, 1, 2),
    1: (64, 8, 3, 2),
    2: (32, 16, 7, 4),
}
_ORDER = (2, 1, 0)  # smallest level first


def _build_nc():
    import concourse.bacc as bacc
    import concourse.tile as tile
    from concourse import mybir

    f32 = mybir.dt.float32
    f32r = mybir.dt.float32r
    gt = mybir.AluOpType.is_gt
    X = mybir.AxisListType.X

    nc = bacc.Bacc("TRN2", target_bir_lowering=False, debug=False)

    feats = {
        0: nc.dram_tensor("feat0", [_C, 128, 128], f32r, kind="ExternalInput"),
        1: nc.dram_tensor("feat1", [_C, 64, 64], f32r, kind="ExternalInput"),
        2: nc.dram_tensor("feat2", [_C, 32, 32], f32r, kind="ExternalInput"),
    }
    scr = nc.dram_tensor("scribbles", [_I, 512, 512], f32, kind="ExternalInput")
    out_d = nc.dram_tensor("out", [_I, 3 * (_C + 1)], f32, kind="ExternalOutput")

    with tile.TileContext(nc) as tc:
        with (
            tc.tile_pool(name="singles", bufs=1) as singles,
            tc.tile_pool(name="scrib", bufs=3) as scrib,
            tc.tile_pool(name="vtmp", bufs=2) as vtmp,
            tc.tile_pool(name="srtmp", bufs=2) as srtmp,
            tc.tile_pool(name="mtmp", bufs=3) as mtmpp,
            tc.tile_pool(name="fmap", bufs=2) as fpool,
            tc.tile_pool(name="psum", bufs=2, space="PSUM") as psum,
        ):
            ones = singles.tile([128, 1], f32, tag="ones")
            nc.vector.memset(ones[:], 1.0)
            stag = singles.tile([_I, 3 * (_C + 1)], f32, tag="stag")

            def make_masks(li):
                """Scribble loads (sync ring) + DVE resize -> mask tile m."""
                h, k, off, ipack = _LEVELS[li]
                w = h
                m = singles.tile([h, _I, w], f32r, tag=f"m{li}")
                for t in range(_I // ipack):
                    i0 = t * ipack
                    # rows (k*y+off, k*y+off+1) are adjacent -> merged 4KiB runs
                    seng = nc.sync if t % 2 == 0 else nc.gpsimd
                    if li == 0:
                        # partitions = y(128); free = (i-pair, row-pair * x)
                        st = scrib.tile([128, ipack, 1024], f32, tag="st")
                        seng.dma_start(
                            out=st[:],
                            in_=scr[i0 : i0 + ipack]
                            .rearrange("i (y k) x -> y i k x", k=k)[
                                :, :, off : off + 2, :
                            ]
                            .rearrange("y i k x -> y i (k x)"),
                        )
                        for il in range(ipack):
                            v = vtmp.tile([128, 512], f32, tag="v")
                            nc.vector.tensor_add(
                                v[:], st[:, il, 0:512], st[:, il, 512:1024]
                            )
                            vk = v[:].rearrange("p (x k) -> p x k", k=k)
                            sr = srtmp.tile([128, w], f32, tag="sr")
                            nc.vector.tensor_add(
                                sr[:], vk[:, :, off], vk[:, :, off + 1]
                            )
                            nc.vector.tensor_scalar(
                                out=m[:, i0 + il, :], in0=sr[:], scalar1=2.0,
                                scalar2=None, op0=gt,
                            )
                    else:
                        # partitions = (i-sub, y); one mask tile per pack,
                        # repacked per-image into m via tiny SBUF->SBUF DMAs
                        st = scrib.tile([128, 1, 1024], f32, tag="st")
                        seng.dma_start(
                            out=st[:, 0, :].rearrange("p (k x) -> p k x", k=2),
                            in_=scr[i0 : i0 + ipack].rearrange(
                                "i (y k) x -> i y k x", k=k
                            )[:, :, off : off + 2, :],
                        )
                        v = vtmp.tile([128, 512], f32, tag="v")
                        nc.vector.tensor_add(v[:], st[:, 0, 0:512], st[:, 0, 512:1024])
                        vk = v[:].rearrange("p (x k) -> p x k", k=k)
                        sr = srtmp.tile([128, w], f32, tag="sr")
                        nc.vector.tensor_add(sr[:], vk[:, :, off], vk[:, :, off + 1])
                        mt = mtmpp.tile([128, w], f32r, tag="mt")
                        nc.vector.tensor_scalar(
                            out=mt[:], in0=sr[:], scalar1=2.0, scalar2=None, op0=gt
                        )
                        for ii in range(ipack):
                            nc.gpsimd.dma_start(
                                out=m[:, i0 + ii, :],
                                in_=mt[ii * h : (ii + 1) * h, :],
                            )
                return m

            def load_fmap(li, x0, wx, split=False):
                """One fmap tile [h, C, wx] covering x in [x0, x0+wx)."""
                h = _LEVELS[li][0]
                f = fpool.tile([h, _C, wx], f32r, tag="f")
                for g in range(_C // _CG):
                    feng = (nc.sync if g % 2 == 0 else nc.scalar) if split else nc.scalar
                    feng.dma_start(
                        out=f[:, g * _CG : (g + 1) * _CG, :],
                        in_=feats[li][g * _CG : (g + 1) * _CG][
                            :, :, x0 : x0 + wx
                        ].rearrange("c y x -> y c x"),
                    )
                return f

            def level_matmuls(li, m, ftiles, acc):
                """One N=256 fp32r matmul per pixel column x."""
                h = _LEVELS[li][0]
                w = h
                xi = 0
                for f, x0, wx in ftiles:
                    for xl in range(wx):
                        nc.tensor.matmul(
                            acc[:],
                            m[:, :, x0 + xl],
                            f[:, :, xl],
                            start=(xi == 0),
                            stop=(xi == w - 1),
                        )
                        xi += 1

            def finish_level(li, m, acc, slot):
                h = _LEVELS[li][0]
                r = singles.tile([h, _I], f32, tag=f"r{li}")
                nc.vector.reduce_sum(out=r[:], in_=m[:].bitcast(f32), axis=X)
                cntp = psum.tile([_I, 1], f32, tag="cntp")
                nc.tensor.matmul(cntp[:], r[:], ones[:h, :], start=True, stop=True)
                base = slot * (_C + 1)
                nc.vector.tensor_copy(stag[:, base : base + _C], acc[:])
                nc.vector.tensor_copy(stag[:, base + _C : base + _C + 1], cntp[:])

            # ---- emission: small levels first, L0 split into two x-chunks ----
            m2 = make_masks(2)
            m1 = make_masks(1)
            m0 = make_masks(0)

            f2 = load_fmap(2, 0, 32)
            f1 = load_fmap(1, 0, 64)
            f0a = load_fmap(0, 0, 64)
            f0b1 = load_fmap(0, 64, 32, split=True)
            f0b2 = load_fmap(0, 96, 32, split=True)

            acc2 = psum.tile([_I, _C], f32, tag="acc")
            level_matmuls(2, m2, [(f2, 0, 32)], acc2)
            finish_level(2, m2, acc2, 2)

            acc1 = psum.tile([_I, _C], f32, tag="acc")
            level_matmuls(1, m1, [(f1, 0, 64)], acc1)
            finish_level(1, m1, acc1, 1)

            acc0 = psum.tile([_I, _C], f32, tag="acc")
            level_matmuls(0, m0, [(f0a, 0, 64), (f0b1, 64, 32), (f0b2, 96, 32)], acc0)
            finish_level(0, m0, acc0, 0)

            nc.gpsimd.dma_start(out=out_d[:], in_=stag[:])

    nc.compile()
    return nc


def _host_fallback(scr_bi, fmap_b, h, k, off):
    """Feature at argmax of the soft mask; only used when a mask is empty."""
    V = scr_bi[off::k, :][:h].astype(np.float32) + scr_bi[off + 1 :: k, :][:h]
    sr4 = V[:, off::k][:, :h] + V[:, off + 1 :: k][:, :h]
    idx = int(np.argmax(np.float32(0.25) * sr4))
    y, x = divmod(idx, h)
    return fmap_b[:, y, x]


def kernel(feat0, feat1, feat2, scribbles):
    import sys

    for p in ("/opt/trn_rl_repo", "/opt/pypackages"):
        if p not in sys.path:
            sys.path.append(p)
    from concourse.bass_utils import run_bass_kernel_spmd

    feat0 = np.asarray(feat0, dtype=np.float32)
    feat1 = np.asarray(feat1, dtype=np.float32)
    feat2 = np.asarray(feat2, dtype=np.float32)
    scribbles = np.asarray(scribbles, dtype=np.float32)

    nc = _build_nc()
    in_maps = [
        {
            "feat0": np.ascontiguousarray(feat0[b]),
            "feat1": np.ascontiguousarray(feat1[b]),
            "feat2": np.ascontiguousarray(feat2[b]),
            "scribbles": np.ascontiguousarray(scribbles[b]),
        }
        for b in range(_B)
    ]
    res = run_bass_kernel_spmd(nc, in_maps, core_ids=list(range(_B)))
    raw = np.stack([res.results[b]["out"] for b in range(_B)])  # [B, I, 3*257]
    raw = raw.reshape(_B, _I, 3, _C + 1)
    ssum = raw[..., :_C].astype(np.float32)  # [B, I, 3, C]
    cnt = raw[..., _C].astype(np.float32)  # [B, I, 3]

    mean = ssum / np.maximum(cnt, np.float32(1.0))[..., None]

    if (cnt == 0).any():  # never for non-degenerate inputs
        fm = [feat0, feat1, feat2]
        for b, i, li in zip(*np.nonzero(cnt == 0)):
            h, k, off, _ = _LEVELS[li]
            mean[b, i, li] = _host_fallback(scribbles[b, i], fm[li][b], h, k, off)

    out = (mean[:, :, 0] + mean[:, :, 1] + mean[:, :, 2]) / np.float32(3.0)
    return out.astype(np.float32)
